# revision 1
# baseline (speedup 1.0000x reference)
"""Trainium2 Bass kernel for the Burgers PINN problem.

Computes u(x) for IC/BC points and the PDE residual u_t + u*u_x - nu*u_xx
for collocation points, where u is a tanh MLP (2 -> 128 -> ... -> 1, 7
hidden-to-hidden layers).

Strategy:
  - Pure data parallelism: every core gets 1/8 of x_f AND 1/8 of each
    IC/BC set (17408 points per core); MLP weights are replicated.
  - Derivatives via forward-mode Taylor propagation of four streams
    (H, X=+-Hx, Y=+-Ht, Z=Hxx) in transposed layout [features, points]:
        A    = W^T H_prev                       (PE, fp16)
        H    = tanh(A + b)                      (ACT -> fp16)
        S2   = 2*AX^2 = Square(sqrt2*AX)        (ACT -> fp16)
        s    = H^2                              (ACT Square)
        g    = s - 1 = -tanh'                   (DVE ts, fp16 4x mode)
        X|Y  = g (.) [AX|AY]                    (DVE TT bcast, PSUM op)
        m    = H (.) S2                         (Pool TT)
        T    = m - AZ                           (DVE TT, PSUM op)
        Z    = g (.) T  ( = +Hxx exactly )      (Pool/DVE TT, per layer)
  - All stream tensors are fp16 in SBUF: DVE tensor_scalar ops hit the
    4x perf mode (194ns/512col) and SBUF-only TT the 2x mode (327ns);
    matmul weights fp16 (1 PE cycle/row, same rate as f32r).  PSUM
    stays fp32 (TRN2).
  - HW restricts GPSIMD (Pool) to SBUF-only TensorTensor, so the
    PSUM-consuming elementwise ops live on DVE and Pool takes m plus
    most Z layers; one Z layer rides DVE to balance (Z_ON_DVE_LAYERS).
  - Final layer packs [u; +-ux; +-ut; uxx] into one [4, T] PSUM tile via
    four accumulating matmuls with sparse [128,4] lhsT columns.
  - Residual combination + concat on host (tiny); X/Y sign parity after
    7 hidden layers is +1, so no host-side sign fix is needed.
"""

import sys

if "/opt/trn_rl_repo" not in sys.path:
    sys.path.insert(0, "/opt/trn_rl_repo")

import numpy as np

N_CORES = 8
H = 128
L = 7  # hidden-to-hidden layers
NF, N0, NB = 131072, 4096, 2048
NF_C, N0_C, NB_C = NF // N_CORES, N0 // N_CORES, NB // N_CORES
NPTS = N0_C + 2 * NB_C + NF_C  # 17408 points per core
TILE = 512
NTILES = NPTS // TILE  # 34
NU = 0.01 / np.pi

# consts tensor layout (columns of a [128, NCONST] fp32 array)
IBH = 0            # cols 0..6   : b_hid[l]
IB_IN = 7          # col  7      : b_in
IAX = 8            # col  8      : W_in[0, :]        (d a0/dx per partition)
IAT = 9            # col  9      : W_in[1, :]        (d a0/dt per partition)
IAX2 = 10          # col 10      : 2 * W_in[0,:]^2
IWF = 11           # cols 11..26 : four [128,4] lhsT mats, mat m has W_out in col m
NCONST = 27

# layers whose s = H^2 runs on Pool (TT) instead of ACT (Square), to balance
S_ON_POOL_LAYERS = ()
# layers whose Z = g*t runs on DVE instead of Pool, to balance
Z_ON_DVE_LAYERS = (3,)

TRACE = False
LAST_RESULTS = None

_CACHE = {}


def _build_bass():
    import concourse.tile as tile
    from concourse import bacc, mybir

    f32 = mybir.dt.float32
    f32r = mybir.dt.float32r
    f16 = mybir.dt.float16
    AF = mybir.ActivationFunctionType
    OP = mybir.AluOpType
    SQRT2 = float(np.sqrt(2.0))

    nc = bacc.Bacc("TRN2", target_bir_lowering=False,
                   detect_race_conditions=False)

    xT = nc.dram_tensor("xt", [2, NPTS], f32, kind="ExternalInput")
    whid = nc.dram_tensor("whid", [L, H, H], f32, kind="ExternalInput")
    win = nc.dram_tensor("win", [2, H], f32, kind="ExternalInput")
    consts = nc.dram_tensor("consts", [H, NCONST], f32, kind="ExternalInput")
    out4 = nc.dram_tensor("out4", [4, NPTS], f32, kind="ExternalOutput")

    with tile.TileContext(nc) as tc:
        with (
            tc.tile_pool(name="wpool", bufs=1) as wp,
            tc.tile_pool(name="spool", bufs=8) as sp,
            tc.tile_pool(name="tpool", bufs=8) as tp,
            tc.tile_pool(name="ppool", bufs=1, space="PSUM") as pp,
        ):
            w_f = wp.tile([H, L * H], f32, tag="whidf")
            for l in range(L):
                nc.sync.dma_start(w_f[:, l * H:(l + 1) * H], whid[l, :, :])
            win_sb = wp.tile([2, H], f32, tag="win")
            nc.sync.dma_start(win_sb[:, :], win[:, :])
            c_sb = wp.tile([H, NCONST], f32, tag="consts")
            nc.sync.dma_start(c_sb[:, :], consts[:, :])

            # one-time conversion of matmul weights to fp16
            w_r = wp.tile([H, L * H], f16, tag="whidr")
            nc.vector.tensor_copy(w_r[:, :], w_f[:, :])
            wfin_r = wp.tile([H, 16], f16, tag="wfinr")
            nc.vector.tensor_copy(wfin_r[:, :], c_sb[:, IWF:IWF + 16])

            def col(j):
                return c_sb[:, j:j + 1]

            # tiles 0,1 hold the 1024 IC/BC points: forward pass only.
            AUX_TILES = (N0_C + 2 * NB_C) // TILE  # = 2
            STRIDE = (5,)  # launch-gap pattern between consecutive tiles
            state = {}
            parity = {}

            def stage_in(i):
                """Input layer (f32r matmul, K=2) + layer-0 streams."""
                tsl = slice(i * TILE, (i + 1) * TILE)
                r = parity[i]
                aux = i < AUX_TILES
                x_t = sp.tile([2, TILE], f32, tag="xin")
                nc.sync.dma_start(x_t[:, :], xT[:, tsl])
                a = pp.tile([H, TILE], f32, tag=f"pa{r}")
                nc.tensor.matmul(a[:, :], win_sb[:, :], x_t[:, :],
                                 start=True, stop=True)
                yield
                h = sp.tile([H, TILE], f16, tag="h")
                nc.scalar.activation(h[:, :], a[:, :], AF.Tanh, bias=col(IB_IN))
                xy = z = None
                if not aux:
                    yield
                    s = tp.tile([H, TILE], f16, tag="s")
                    nc.scalar.square(s[:, :], h[:, :])
                    yield
                    g = tp.tile([H, TILE], f16, tag="g")
                    nc.vector.tensor_scalar_sub(g[:, :], s[:, :], 1.0)  # H^2-1
                    yield
                    xy = sp.tile([H, 2, TILE], f16, tag="xy")
                    nc.vector.tensor_scalar_mul(xy[:, 0, :], g[:, :], col(IAX))
                    nc.vector.tensor_scalar_mul(xy[:, 1, :], g[:, :], col(IAT))
                    yield
                    tq = tp.tile([H, TILE], f16, tag="tq")
                    nc.vector.tensor_scalar_mul(tq[:, :], g[:, :], col(IAX2))
                    yield
                    z = sp.tile([H, TILE], f16, tag="z")
                    nc.gpsimd.tensor_mul(z[:, :], tq[:, :], h[:, :])
                state[i] = (h, xy, z)

            def stage_hidden(i, l):
                r = parity[i]
                aux = i < AUX_TILES
                h, xy, z = state[i]
                wl = w_r[:, l * H:(l + 1) * H]
                a = pp.tile([H, TILE], f32, tag=f"pa{r}")
                nc.tensor.matmul(a[:, :], wl, h[:, :], start=True, stop=True)
                if not aux:
                    axy = pp.tile([H, 2, TILE], f32, tag=f"paxy{r}")
                    az = pp.tile([H, TILE], f32, tag=f"paz{r}")
                    nc.tensor.matmul(axy[:, 0, :], wl, xy[:, 0, :],
                                     start=True, stop=True)
                    yield
                    nc.tensor.matmul(axy[:, 1, :], wl, xy[:, 1, :],
                                     start=True, stop=True)
                    nc.tensor.matmul(az[:, :], wl, z[:, :],
                                     start=True, stop=True)
                yield
                hn = sp.tile([H, TILE], f16, tag="h")
                nc.scalar.activation(hn[:, :], a[:, :], AF.Tanh,
                                     bias=col(IBH + l))
                xyn = zn = None
                if not aux:
                    yield
                    s2 = tp.tile([H, TILE], f16, tag="s2")
                    nc.scalar.activation(s2[:, :], axy[:, 0, :], AF.Square,
                                         scale=SQRT2)  # = 2*AX^2
                    yield
                    s = tp.tile([H, TILE], f16, tag="s")
                    if l in S_ON_POOL_LAYERS:
                        nc.gpsimd.tensor_mul(s[:, :], hn[:, :], hn[:, :])
                    else:
                        nc.scalar.square(s[:, :], hn[:, :])
                    yield
                    g = tp.tile([H, TILE], f16, tag="g")
                    nc.vector.tensor_scalar_sub(g[:, :], s[:, :], 1.0)
                    yield
                    xyn = sp.tile([H, 2, TILE], f16, tag="xy")
                    g_b = g[:, :].unsqueeze(1).broadcast_to([H, 2, TILE])
                    nc.vector.tensor_tensor(xyn[:, :, :], axy[:, :, :], g_b,
                                            OP.mult)
                    yield
                    m = tp.tile([H, TILE], f16, tag="m")
                    nc.gpsimd.tensor_mul(m[:, :], hn[:, :], s2[:, :])
                    yield
                    t = tp.tile([H, TILE], f16, tag="t")
                    nc.vector.tensor_sub(t[:, :], m[:, :], az[:, :])
                    yield
                    zn = sp.tile([H, TILE], f16, tag="z")
                    if l in Z_ON_DVE_LAYERS:
                        nc.vector.tensor_mul(zn[:, :], g[:, :], t[:, :])
                    else:
                        nc.gpsimd.tensor_mul(zn[:, :], g[:, :], t[:, :])
                state[i] = (hn, xyn, zn)

            def stage_out(i):
                tsl = slice(i * TILE, (i + 1) * TILE)
                r = parity[i]
                aux = i < AUX_TILES
                h, xy, z = state.pop(i)
                o = pp.tile([4, TILE], f32, tag=f"pa{r}")
                if aux:
                    nc.tensor.matmul(o[:, :], wfin_r[:, 0:4], h[:, :],
                                     start=True, stop=True)
                else:
                    rhss = [h[:, :], xy[:, 0, :], xy[:, 1, :], z[:, :]]
                    for mi, rhs in enumerate(rhss):
                        nc.tensor.matmul(o[:, :], wfin_r[:, 4 * mi:4 * (mi + 1)],
                                         rhs, start=(mi == 0), stop=(mi == 3))
                yield
                o_t = sp.tile([4, TILE], f32, tag="ot")
                nc.scalar.copy(o_t[:, :], o[:, :])
                nc.sync.dma_start(out4[:, tsl], o_t[:, :])

            def tile_gen(i):
                """All stages of tile i; yields mark slot boundaries."""
                yield from stage_in(i)
                yield "stage"
                for l in range(L):
                    yield from stage_hidden(i, l)
                    if l < L - 1:
                        yield "stage"
                yield from stage_out(i)
                yield "stage"

            # software-pipelined wavefront: tile i starts STRIDE stages after
            # tile i-1; ops of the active tiles are emitted round-robin so the
            # in-order engine queues interleave the two chains.
            # launch order: full tiles first, aux (short) tiles last so the
            # pipeline fills with real work and aux fills the drain.
            order = list(range(AUX_TILES, NTILES)) + list(range(AUX_TILES))
            starts = {}
            t0 = 0
            for k, i in enumerate(order):
                parity[i] = k % 2
                starts[i] = t0
                t0 += STRIDE[k % len(STRIDE)]

            gens = []
            next_k = 0
            slot = 0
            while gens or next_k < NTILES:
                while next_k < NTILES and starts[order[next_k]] <= slot:
                    gens.append(tile_gen(order[next_k]))
                    next_k += 1
                pending = list(gens)
                while pending:
                    for gn in list(pending):
                        tok = next(gn, "done")
                        if tok == "stage":
                            pending.remove(gn)
                        elif tok == "done":
                            pending.remove(gn)
                            gens.remove(gn)
                slot += 1

    nc.compile()
    return nc


def _get_nc():
    if "nc" not in _CACHE:
        _CACHE["nc"] = _build_bass()
    return _CACHE["nc"]


def kernel(x_f, x0_cat, xb_left_cat, xb_right_cat,
           W_in, b_in, W_hid, b_hid, W_out, b_out):
    global LAST_RESULTS
    from concourse.bass_utils import run_bass_kernel_spmd

    f32 = np.float32
    x_f = np.asarray(x_f, f32)
    x0_cat = np.asarray(x0_cat, f32)
    xb_left_cat = np.asarray(xb_left_cat, f32)
    xb_right_cat = np.asarray(xb_right_cat, f32)
    W_in = np.ascontiguousarray(np.asarray(W_in, f32))
    b_in = np.asarray(b_in, f32)
    W_hid = np.ascontiguousarray(np.asarray(W_hid, f32))
    b_hid = np.asarray(b_hid, f32)
    W_out = np.asarray(W_out, f32)
    b_out = np.asarray(b_out, f32)

    consts = np.zeros((H, NCONST), f32)
    consts[:, 0:L] = b_hid.T
    consts[:, IB_IN] = b_in
    consts[:, IAX] = W_in[0]
    consts[:, IAT] = W_in[1]
    consts[:, IAX2] = 2.0 * W_in[0] ** 2
    for mi in range(4):
        consts[:, IWF + 4 * mi + mi] = W_out[:, 0]
    consts = np.ascontiguousarray(consts)

    in_maps = []
    for k in range(N_CORES):
        pts = np.concatenate([
            x0_cat[k * N0_C:(k + 1) * N0_C],
            xb_left_cat[k * NB_C:(k + 1) * NB_C],
            xb_right_cat[k * NB_C:(k + 1) * NB_C],
            x_f[k * NF_C:(k + 1) * NF_C],
        ], axis=0)  # [NPTS, 2]
        in_maps.append({
            "xt": np.ascontiguousarray(pts.T),
            "whid": W_hid,
            "win": W_in,
            "consts": consts,
        })

    nc = _get_nc()
    res = run_bass_kernel_spmd(nc, in_maps, core_ids=list(range(N_CORES)),
                               trace=TRACE)
    LAST_RESULTS = res

    u0_parts, ubl_parts, ubr_parts, r_parts = [], [], [], []
    for k in range(N_CORES):
        o = res.results[k]["out4"]  # [4, NPTS]
        u = o[0] + b_out[0]
        # X/Y parity after 7 hidden layers is +1 (see module docstring)
        ux, ut, uxx = o[1], o[2], o[3]
        u0_parts.append(u[:N0_C])
        ubl_parts.append(u[N0_C:N0_C + NB_C])
        ubr_parts.append(u[N0_C + NB_C:N0_C + 2 * NB_C])
        f = slice(N0_C + 2 * NB_C, None)
        r_parts.append(ut[f] + u[f] * ux[f] - NU * uxx[f])

    out = np.concatenate(u0_parts + ubl_parts + ubr_parts + r_parts)
    return np.ascontiguousarray(out.reshape(-1, 1).astype(f32))



# revision 3
# speedup vs baseline: 1.0448x; 1.0448x over previous
"""Trainium2 Bass kernel for the Burgers PINN problem (v2).

Computes u(x) for IC/BC points and the PDE residual u_t + u*u_x - nu*u_xx
for collocation points, where u is a tanh MLP (2 -> 128 -> ... -> 1, 7
hidden-to-hidden layers).

Strategy (v2 — engine-load rebalance of the v1 Taylor kernel):
  - Pure data parallelism: every core gets 1/8 of x_f AND 1/8 of each
    IC/BC set (17408 points per core); MLP weights replicated, host
    pre-converted to fp16.
  - Forward-mode Taylor streams H, X (.)= +-u_x, Y = +-u_t, Z = +-u_xx in
    transposed layout [features, points], packed per layer into ONE
    [128, 4, T] fp16 SBUF tile so stream matmuls read contiguous slices.
  - Per hidden layer:
      a|ax|ay|az = W^T [H|X|Y|Z]     (4 PE matmuls, fp16, separate banks)
      H    = tanh(a + b)             (ACT)
      s2   = 2*AX^2 = Square(sqrt2*ax)  (ACT, psum->f16)
      s    = H^2                     (ACT Square | DVE TT 2x | Pool TT)
      m    = H (.) s2                (Pool TT | DVE TT 2x)
      az  += -/+ I m                 (PE accumulating identity matmul ==
                                      t' = az -/+ m computed ON PE)
      X|Y  = (s-1) (.) [ax|ay]       (ONE DVE scalar_tensor_tensor, fused)
      Z    = (s-1) (.) az'           (DVE scalar_tensor_tensor)
    The +-I alternation tracks the Z-stream sign parity tau_l = (-1)^l;
    the final parity is folded into a negated W_out column.  The m-chain
    (s2 -> m -> I-matmul -> z) lags the xy pipeline by a constant offset
    instead of compounding into the layer recurrence (that is why xy and
    z are separate stt ops, not one 3-wide op).
  - Input layer: X0/Y0/tq seeds via two-scalar tensor_scalar
    ((s0-1)*col in one 4x op); z0 = tq (.) H0 on Pool.
  - Output: 4 accumulating sparse matmuls -> [4,T] psum -> ACT copy -> DMA.
  - s/m placement is split ACT/Pool/DVE by static fractions to balance
    engine loads (DVE is the bound: 1192+658 ns of mandatory 1x
    psum-crossing stt work per layer-tile).
"""

import sys

if "/opt/trn_rl_repo" not in sys.path:
    sys.path.insert(0, "/opt/trn_rl_repo")

import numpy as np

N_CORES = 8
H = 128
L = 7  # hidden-to-hidden layers
NF, N0, NB = 131072, 4096, 2048
NF_C, N0_C, NB_C = NF // N_CORES, N0 // N_CORES, NB // N_CORES
NPTS = N0_C + 2 * NB_C + NF_C  # 17408 points per core
TILE = 512
NTILES = NPTS // TILE  # 34
NU = 0.01 / np.pi

# consts tensor layout (columns of a [128, NCONST] fp32 array)
IBH = 0            # cols 0..6   : b_hid[l]
IB_IN = 7          # col  7      : b_in
IAX = 8            # col  8      : W_in[0, :]        (d a0/dx per partition)
IAT = 9            # col  9      : W_in[1, :]        (d a0/dt per partition)
IAX2 = 10          # col 10      : 2 * W_in[0,:]^2
NCONST = 11

# engine placement knobs: for hidden layer (t, l) use key = (t*7+l) % len()
S_SCHED = ("A", "A", "P", "A", "A", "P", "A", "P")   # s = H^2: ACT or Pool
M_SCHED = ("P",)                                     # m = H*s2: Pool or DVE
STRIDE = (5,)

TRACE = False
LAST_RESULTS = None

_CACHE = {}


def _build_bass():
    import concourse.tile as tile
    from concourse import bacc, mybir

    f32 = mybir.dt.float32
    f16 = mybir.dt.float16
    AF = mybir.ActivationFunctionType
    OP = mybir.AluOpType
    SQRT2 = float(np.sqrt(2.0))

    nc = bacc.Bacc("TRN2", target_bir_lowering=False,
                   detect_race_conditions=False)

    xT = nc.dram_tensor("xt", [2, NPTS], f16, kind="ExternalInput")
    whid = nc.dram_tensor("whid", [L, H, H], f16, kind="ExternalInput")
    win = nc.dram_tensor("win", [2, H], f16, kind="ExternalInput")
    iden = nc.dram_tensor("iden", [H, 2 * H], f16, kind="ExternalInput")
    wfin = nc.dram_tensor("wfin", [H, 16], f16, kind="ExternalInput")
    consts = nc.dram_tensor("consts", [H, NCONST], f32, kind="ExternalInput")
    out4 = nc.dram_tensor("out4", [4, NPTS], f32, kind="ExternalOutput")

    with tile.TileContext(nc) as tc:
        with (
            tc.tile_pool(name="wpool", bufs=1) as wp,
            tc.tile_pool(name="spool", bufs=6) as sp,
            tc.tile_pool(name="tpool", bufs=8) as tp,
            tc.tile_pool(name="ppool", bufs=1, space="PSUM") as pp,
        ):
            w_r = wp.tile([H, L * H], f16, tag="whid")
            for l in range(L):
                nc.sync.dma_start(w_r[:, l * H:(l + 1) * H], whid[l, :, :])
            win_sb = wp.tile([2, H], f16, tag="win")
            nc.sync.dma_start(win_sb[:, :], win[:, :])
            i_sb = wp.tile([H, 2 * H], f16, tag="iden")
            nc.sync.dma_start(i_sb[:, :], iden[:, :])
            wfin_sb = wp.tile([H, 16], f16, tag="wfin")
            nc.sync.dma_start(wfin_sb[:, :], wfin[:, :])
            c_sb = wp.tile([H, NCONST], f32, tag="consts")
            nc.sync.dma_start(c_sb[:, :], consts[:, :])

            def col(j):
                return c_sb[:, j:j + 1]

            negI = i_sb[:, 0:H]   # -identity
            posI = i_sb[:, H:2 * H]

            # tiles 0,1 hold the 1024 IC/BC points: forward pass only.
            AUX_TILES = (N0_C + 2 * NB_C) // TILE  # = 2
            state = {}
            parity = {}

            def s_engine(i, l):
                return S_SCHED[(i * 7 + l) % len(S_SCHED)]

            def m_engine(i, l):
                return M_SCHED[(i * 7 + l) % len(M_SCHED)]

            def stage_in(i):
                """Input layer (K=2 fp16 matmul) + layer-0 stream seeds."""
                tsl = slice(i * TILE, (i + 1) * TILE)
                r = parity[i]
                aux = i < AUX_TILES
                x_t = sp.tile([2, TILE], f16, tag="xin")
                nc.sync.dma_start(x_t[:, :], xT[:, tsl])
                a = pp.tile([H, TILE], f32, tag=f"pa{r}")
                nc.tensor.matmul(a[:, :], win_sb[:, :], x_t[:, :],
                                 start=True, stop=True)
                yield
                S = sp.tile([H, 4, TILE], f16, tag="S")
                nc.scalar.activation(S[:, 0, :], a[:, :], AF.Tanh,
                                     bias=col(IB_IN))
                if not aux:
                    yield
                    s = tp.tile([H, TILE], f16, tag="s")
                    eng = s_engine(i, 0)
                    if eng == "A":
                        nc.scalar.square(s[:, :], S[:, 0, :])
                    elif eng == "P":
                        nc.gpsimd.tensor_mul(s[:, :], S[:, 0, :], S[:, 0, :])
                    else:
                        nc.vector.tensor_mul(s[:, :], S[:, 0, :], S[:, 0, :])
                    yield
                    # X0 = (s-1)*Wx, Y0 = (s-1)*Wt, tq = (s-1)*2Wx^2
                    nc.vector.tensor_scalar(S[:, 1, :], s[:, :], 1.0,
                                            col(IAX), OP.subtract, OP.mult)
                    nc.vector.tensor_scalar(S[:, 2, :], s[:, :], 1.0,
                                            col(IAT), OP.subtract, OP.mult)
                    yield
                    tq = tp.tile([H, TILE], f16, tag="tq")
                    nc.vector.tensor_scalar(tq[:, :], s[:, :], 1.0,
                                            col(IAX2), OP.subtract, OP.mult)
                    yield
                    # z0 = tq (.) H0  (= +u_xx stream seed, tau_0 = +1)
                    nc.gpsimd.tensor_mul(S[:, 3, :], tq[:, :], S[:, 0, :])
                state[i] = S
                yield "stage"

            def stage_hidden(i, l):
                r = parity[i]
                aux = i < AUX_TILES
                Sp = state[i]
                wl = w_r[:, l * H:(l + 1) * H]
                a = pp.tile([H, TILE], f32, tag=f"pa{r}")
                nc.tensor.matmul(a[:, :], wl, Sp[:, 0, :], start=True,
                                 stop=True)
                if not aux:
                    pxy = pp.tile([H, 2, TILE], f32, tag=f"pxy{r}")
                    nc.tensor.matmul(pxy[:, 0, :], wl, Sp[:, 1, :],
                                     start=True, stop=True)
                    yield
                    nc.tensor.matmul(pxy[:, 1, :], wl, Sp[:, 2, :],
                                     start=True, stop=True)
                    pz = pp.tile([H, TILE], f32, tag=f"pz{r}")
                    nc.tensor.matmul(pz[:, :], wl, Sp[:, 3, :],
                                     start=True, stop=False)
                yield
                S = sp.tile([H, 4, TILE], f16, tag="S")
                nc.scalar.activation(S[:, 0, :], a[:, :], AF.Tanh,
                                     bias=col(IBH + l))
                if not aux:
                    s2 = tp.tile([H, TILE], f16, tag="s2")
                    nc.scalar.activation(s2[:, :], pxy[:, 0, :], AF.Square,
                                         scale=SQRT2)  # = 2*AX^2
                    yield
                    s = tp.tile([H, TILE], f16, tag="s")
                    eng = s_engine(i, l)
                    if eng == "A":
                        nc.scalar.square(s[:, :], S[:, 0, :])
                    elif eng == "P":
                        nc.gpsimd.tensor_mul(s[:, :], S[:, 0, :], S[:, 0, :])
                    else:
                        nc.vector.tensor_mul(s[:, :], S[:, 0, :], S[:, 0, :])
                    yield
                    # X|Y = (s-1) (.) [ax|ay]   (one fused DVE op)
                    s_b = s[:, :].unsqueeze(1).broadcast_to([H, 2, TILE])
                    nc.vector.scalar_tensor_tensor(
                        S[:, 1:3, :], s_b, 1.0, pxy[:, :, :],
                        OP.subtract, OP.mult)
                    yield
                    m = tp.tile([H, TILE], f16, tag="m")
                    if m_engine(i, l) == "P":
                        nc.gpsimd.tensor_mul(m[:, :], S[:, 0, :], s2[:, :])
                    else:
                        nc.vector.tensor_mul(m[:, :], S[:, 0, :], s2[:, :])
                    yield
                    # az' = az -/+ m on PE; tau_in=+1 on even l -> -I
                    ii = negI if (l % 2 == 0) else posI
                    nc.tensor.matmul(pz[:, :], ii, m[:, :],
                                     start=False, stop=True)
                    yield
                    # Z = (s-1) (.) az'
                    nc.vector.scalar_tensor_tensor(
                        S[:, 3, :], s[:, :], 1.0, pz[:, :],
                        OP.subtract, OP.mult)
                state[i] = S
                yield "stage"

            def stage_out(i):
                tsl = slice(i * TILE, (i + 1) * TILE)
                r = parity[i]
                aux = i < AUX_TILES
                S = state.pop(i)
                o = pp.tile([4, TILE], f32, tag=f"pa{r}")
                if aux:
                    nc.tensor.matmul(o[:, :], wfin_sb[:, 0:4], S[:, 0, :],
                                     start=True, stop=True)
                else:
                    for mi in range(4):
                        nc.tensor.matmul(o[:, :], wfin_sb[:, 4 * mi:4 * (mi + 1)],
                                         S[:, mi, :], start=(mi == 0),
                                         stop=(mi == 3))
                yield
                o_t = sp.tile([4, TILE], f32, tag="ot")
                nc.scalar.copy(o_t[:, :], o[:, :])
                nc.sync.dma_start(out4[:, tsl], o_t[:, :])
                yield "stage"

            def tile_gen(i):
                yield from stage_in(i)
                for l in range(L):
                    yield from stage_hidden(i, l)
                yield from stage_out(i)

            # software-pipelined wavefront (same machinery as v1)
            order = list(range(AUX_TILES, NTILES)) + list(range(AUX_TILES))
            starts = {}
            t0 = 0
            for k, i in enumerate(order):
                parity[i] = k % 2
                starts[i] = t0
                t0 += STRIDE[k % len(STRIDE)]

            gens = []
            next_k = 0
            slot = 0
            while gens or next_k < NTILES:
                while next_k < NTILES and starts[order[next_k]] <= slot:
                    gens.append(tile_gen(order[next_k]))
                    next_k += 1
                pending = list(gens)
                while pending:
                    for gn in list(pending):
                        tok = next(gn, "done")
                        if tok == "stage":
                            pending.remove(gn)
                        elif tok == "done":
                            pending.remove(gn)
                            gens.remove(gn)
                slot += 1

    nc.compile()
    return nc


def _get_nc():
    if "nc" not in _CACHE:
        _CACHE["nc"] = _build_bass()
    return _CACHE["nc"]


def kernel(x_f, x0_cat, xb_left_cat, xb_right_cat,
           W_in, b_in, W_hid, b_hid, W_out, b_out):
    global LAST_RESULTS
    from concourse.bass_utils import run_bass_kernel_spmd

    f32, f16 = np.float32, np.float16
    x_f = np.asarray(x_f, f32)
    x0_cat = np.asarray(x0_cat, f32)
    xb_left_cat = np.asarray(xb_left_cat, f32)
    xb_right_cat = np.asarray(xb_right_cat, f32)
    W_in = np.ascontiguousarray(np.asarray(W_in, f32))
    b_in = np.asarray(b_in, f32)
    W_hid = np.ascontiguousarray(np.asarray(W_hid, f32))
    b_hid = np.asarray(b_hid, f32)
    W_out = np.asarray(W_out, f32)
    b_out = np.asarray(b_out, f32)

    consts = np.zeros((H, NCONST), f32)
    consts[:, 0:L] = b_hid.T
    consts[:, IB_IN] = b_in
    consts[:, IAX] = W_in[0]
    consts[:, IAT] = W_in[1]
    consts[:, IAX2] = 2.0 * W_in[0] ** 2
    consts = np.ascontiguousarray(consts)

    # final sparse matmuls: mat m has +-W_out in col m (Z parity tau_7 = -1)
    wfin = np.zeros((H, 16), f16)
    for mi in range(4):
        sgn = -1.0 if mi == 3 else 1.0
        wfin[:, 4 * mi + mi] = (sgn * W_out[:, 0]).astype(f16)

    iden = np.zeros((H, 2 * H), f16)
    iden[:, 0:H] = -np.eye(H, dtype=f16)
    iden[:, H:2 * H] = np.eye(H, dtype=f16)

    in_maps = []
    for k in range(N_CORES):
        pts = np.concatenate([
            x0_cat[k * N0_C:(k + 1) * N0_C],
            xb_left_cat[k * NB_C:(k + 1) * NB_C],
            xb_right_cat[k * NB_C:(k + 1) * NB_C],
            x_f[k * NF_C:(k + 1) * NF_C],
        ], axis=0)  # [NPTS, 2]
        in_maps.append({
            "xt": np.ascontiguousarray(pts.T.astype(f16)),
            "whid": np.ascontiguousarray(W_hid.astype(f16)),
            "win": np.ascontiguousarray(W_in.astype(f16)),
            "iden": np.ascontiguousarray(iden),
            "wfin": np.ascontiguousarray(wfin),
            "consts": consts,
        })

    nc = _get_nc()
    res = run_bass_kernel_spmd(nc, in_maps, core_ids=list(range(N_CORES)),
                               trace=TRACE)
    LAST_RESULTS = res

    u0_parts, ubl_parts, ubr_parts, r_parts = [], [], [], []
    for k in range(N_CORES):
        o = res.results[k]["out4"]  # [4, NPTS]
        u = o[0] + b_out[0]
        ux, ut, uxx = o[1], o[2], o[3]
        u0_parts.append(u[:N0_C])
        ubl_parts.append(u[N0_C:N0_C + NB_C])
        ubr_parts.append(u[N0_C + NB_C:N0_C + 2 * NB_C])
        f = slice(N0_C + 2 * NB_C, None)
        r_parts.append(ut[f] + u[f] * ux[f] - NU * uxx[f])

    out = np.concatenate(u0_parts + ubl_parts + ubr_parts + r_parts)
    return np.ascontiguousarray(out.reshape(-1, 1).astype(f32))
